# revision 60
# baseline (speedup 1.0000x reference)
"""Full (non-causal) multi-head attention for Trainium2, 8-core SPMD.

Problem: B=4, L=2048, H=16, E=64 fp32.
  scores = einsum('blhe,bshe->bhls', Q, K) * 1/sqrt(E)
  attn   = softmax(scores, axis=-1)
  out    = einsum('bhls,bshd->blhd', attn, V)

Sharding: the 64 (b,h) pairs are split over 8 NeuronCores, 8 pairs per
core; attention is fully independent per (b,h), so no cross-core
communication.  The host hands each core Q^T/K^T/V already transposed /
cast to bf16, and takes back two unnormalized partial accumulators
O'_A/O'_B[65, l] per pair (each with a softmax-denominator ride-along
row), merging + dividing + final transpose on the host.

Per-core algorithm — one fully continuous global pipeline over all 32
(pair, l-pass) units (l in passes of 512, one PSUM bank of fp32):
  - Global burst g runs the QK+exp of unit g//8 (burst g%8 = s-chunks
    2m, 2m+1) and the AV matmuls of the unit DLAG=4 bursts back; the
    AV stream crosses pass and pair boundaries so the PE never drains.
  - Scores are computed transposed, S^T[s, l]: per burst one PSUM tile
    [128, 2, 512] (2 banks); the two QK matmuls (chunk 2m -> PE rows
    0-63, chunk 2m+1 -> rows 64-127, contraction E=64, tile_position
    row tiling) run concurrently and drain into the two banks.
  - exp() runs once per burst as a single FD=1024 instruction over the
    whole [128, 1024] score tile (amortizes the ~150-310-cycle fixed
    per-instruction engine overhead): ScalarE bursts use exact exp
    (activation, bf16 out), VectorE bursts a mean-centered Schraudolph
    bit-trick exp (i16 = round(score*A + B), bitcast bf16; sigma ~1.8%
    per element, zero-mean, common mode cancels in the softmax
    divide).  Bursts STRICTLY alternate engines (DVE odd, ACT even):
    both engines run at ~95% of the PE's 648ns/burst cadence, and any
    same-engine adjacency cascades into PE stalls through the 3-tile
    score ring.
  - AV is row-tiled like QK so every LDWEIGHTS in the kernel is a
    64-row load that background-loads behind the other row-half's
    streaming matmul (a full-array AV LDWEIGHTS cannot overlap
    row-tiled QK streams, which cost ~220ns per QK<->AV transition in
    the v1 layout): chunk a's AV is two concurrent matmuls,
    V'[s 0:64]^T E -> oacc[:,0,:] and V'[s 64:128]^T E -> oacc[:,1,:],
    accumulated over all 16 chunks.  V' carries a ones column so row
    64 of each accumulator half is that half's softmax-denominator
    partial; the host adds the halves.  Each pass's first two AV
    bursts are deferred two slots (burst 2 catches up) so the previous
    pass's drain copy clears the oacc WAR with ~2 bursts of slack.
  - PSUM budget (the binding constraint): 3 score tiles x 2 banks +
    oacc (2 banks) = 8 banks = all of PSUM.
  - Dummy warmup matmuls at kernel start keep the PE HAM activity
    window busy during the first DMA wait so real matmuls start at
    2.4 GHz instead of 1.2 GHz.
"""

import math

import numpy as np
import ml_dtypes
from contextlib import ExitStack

import concourse.bass as bass
import concourse.mybir as mybir
import concourse.tile as tile
from concourse import bacc
from concourse.bass_utils import run_bass_kernel_spmd

N_CORES = 8
B, L, H, E = 4, 2048, 16, 64
PAIRS = (B * H) // N_CORES    # 8 (b,h) pairs per core
P = 128                       # s-chunk size / partition count
NCHUNK = L // P               # 16 s-chunks
LQ = 512                      # l-quarter (one PSUM bank of fp32)
NPASS = L // LQ               # 4 passes over l per pair
NBURST = NCHUNK // 2          # 8 QK bursts (2 chunks each) per pass
SCALE = 1.0 / 8.0             # 1/sqrt(E)
DLAG = 4                      # AV stream trails QK stream by this many bursts
N_WARMUP = 32

F32 = mybir.dt.float32
F16 = mybir.dt.float16
BF16 = mybir.dt.bfloat16
I16 = mybir.dt.int16

# Schraudolph constants: i16 = round(score*SA + SB) bitcast bf16
# approximates exp(score/8) with zero-mean multiplicative error.
LOG2E = 1.4426950408889634
SA = 128.0 * LOG2E * SCALE
_fs = np.linspace(0.0, 1.0, 200001)[:-1]
_ECORR = float(np.mean((1.0 + _fs) * 2.0 ** (-_fs)))
SB = 127.0 * 128.0 - 128.0 * math.log2(_ECORR)

# Engine split, tuned so ScalarE and VectorE each stay under the PE's
# burst cadence: DVE handles global bursts with g % 16 in DVE_BURSTS
# (exp via Schraudolph; ~1221ns/burst vs ScalarE's ~1023), ScalarE the
# rest (exact exp).  18/14 per pair keeps both engines just under the
# PE's ~648ns/burst cadence including their half of the drain copies.
DVE_BURSTS = (1, 3, 5, 7, 9, 11, 13, 15)


def _attention(tc: tile.TileContext, o, qt, kt_d, v, qt0, kt0, v0):
    nc = tc.nc
    EXPF = mybir.ActivationFunctionType.Exp

    with ExitStack() as ctx:
        # PE warmup: dummy matmuls on a zero tile keep the HAM activity
        # window busy while the first pair's DMAs land.  wsb stays open for
        # the whole kernel: if it closed, the io pool would reuse its
        # released SBUF zone (stack allocator overlap-dep) and the input
        # DMAs would inherit a wait on all warmup matmuls.
        wsb = ctx.enter_context(tc.tile_pool(name="wsb", bufs=1))
        wz = wsb.tile([64, 64], BF16, tag="wz")
        nc.vector.memset(wz[:], 0.0)
        with tc.tile_pool(name="wps", bufs=1, space="PSUM") as wps:
            wp = wps.tile([64, 64], F32, tag="wp")
            for _ in range(N_WARMUP):
                nc.tensor.matmul(wp[:], wz[:], wz[:], start=True, stop=True)

        io = ctx.enter_context(tc.tile_pool(name="io", bufs=2))
        etp = ctx.enter_context(tc.tile_pool(name="etp", bufs=8))
        osb = ctx.enter_context(tc.tile_pool(name="osb", bufs=4))

        # PSUM: 3 score tiles x 2 banks + oaccA/oaccB x 1 bank = 8
        pscore = ctx.enter_context(tc.tile_pool(name="pscore", bufs=3, space="PSUM"))
        pacc = ctx.enter_context(tc.tile_pool(name="pacc", bufs=1, space="PSUM"))

        # ---- fully continuous global pipeline ----
        # One burst stream over all (pair, pass) units: global burst g
        # runs the QK+exp of unit g//8, burst g%8, and the AVs of the
        # unit DLAG bursts back.  The AV stream crosses pass and pair
        # boundaries, so the PE never drains at a boundary; the only
        # per-pass serialization is the oacc drain copy (split A/B
        # across both engines so the WAR clears in ~700ns).
        def emit_pair_io(p):
            # load Q^T (duplicated to both halves), K^T (paired), V.
            # The sync engine runs ahead of the PE, so these DMAs issue
            # as soon as the previous-previous pair's reads release the
            # io slot — mid-way through pair p-1.
            qtd = io.tile([P, L], BF16, tag="qtd")
            kt = io.tile([P, NBURST, P], BF16, tag="kt")
            vp = io.tile([P, NCHUNK, E + 1], BF16, tag="vp")
            # kt_d[p] is [2, 8, 64, 128]: half h holds chunks 2c+h.
            # Pair 0 loads from pre-staged contiguous copies (qt0/kt0/
            # v0) so the startup DMA is burst-friendly — safe only here
            # because no matmuls stream during it (contiguous DMA bursts
            # steal SBUF bandwidth from the PE); later pairs keep the
            # strided source patterns, which self-throttle.
            if p == 0:
                nc.sync.dma_start(out=qtd[:], in_=qt0)
                nc.sync.dma_start(out=kt[:], in_=kt0)
                nc.sync.dma_start(out=vp[:], in_=v0)
            else:
                nc.sync.dma_start(out=qtd[0:E, :], in_=qt[p])
                nc.sync.dma_start(out=qtd[E:P, :], in_=qt[p])
                nc.sync.dma_start(
                    out=kt[0:E, :, :],
                    in_=kt_d[p, 0].rearrange("c e l -> e c l"),
                )
                nc.sync.dma_start(
                    out=kt[E:P, :, :],
                    in_=kt_d[p, 1].rearrange("c e l -> e c l"),
                )
                # v[p] is [L, E+1], ones column pre-filled on the host (a
                # device memset would RMW-race the DMA in shared 4B words)
                nc.sync.dma_start(
                    out=vp[:], in_=v[p].rearrange("(c p) e -> p c e", p=P)
                )
            return qtd, kt, vp

        NUNIT = PAIRS * NPASS          # 32 (pair, pass) units
        TOTAL = NUNIT * NBURST         # 256 global bursts
        tiles = {}                     # pair -> (qtd, kt, vp)
        state = {}                     # unit -> (oacc, ets)
        for g in range(TOTAL + DLAG):
            # AV (lagged stream): chunk a = two concurrent row-tiled
            # matmuls accumulating into oacc[:,0,:] / [:,1,:].
            ga = g - DLAG
            if ga >= 0:
                u0, m0 = divmod(ga, NBURST)
                # defer each pass's first two AV bursts by two slots so
                # the previous pass's oacc drain copy gets ~2 bursts of
                # slack before the WAR write (AV a=0, start=True) lands;
                # burst 2 catches up with all three.  Total PE work per
                # pass is unchanged — only the order shifts.
                if m0 in (0, 1):
                    av_bursts = []
                elif m0 == 2:
                    av_bursts = [ga - 2, ga - 1, ga]
                else:
                    av_bursts = [ga]
            else:
                av_bursts = []
            for gaa in av_bursts:
                u0, m0 = divmod(gaa, NBURST)
                p0, q0 = divmod(u0, NPASS)
                (oaccA0, oaccB0), ets0 = state[u0]
                vp0 = tiles[p0][2]
                for a in (2 * m0, 2 * m0 + 1):
                    et, half = ets0[a]
                    nc.tensor.matmul(
                        oaccA0[:], vp0[0:E, a, :], et[0:E, half, :],
                        start=(a == 0), stop=(a == NCHUNK - 1),
                        tile_position=(0, 0),
                    )
                    nc.tensor.matmul(
                        oaccB0[:], vp0[E:P, a, :], et[E:P, half, :],
                        start=(a == 0), stop=(a == NCHUNK - 1),
                        tile_position=(E, 0),
                    )
                if m0 == NBURST - 1:
                    # drain the finished pass's accumulators: split A/B
                    # across both engines (separate tiles — one shared
                    # tile would add a false cross-engine WAW) so the
                    # WAR on oacc clears in one FD=512 copy (~700ns);
                    # fp16 halves the DMA bytes.  Host adds + divides.
                    osumA = osb.tile([E + 1, LQ], F16, tag="osumA")
                    osumB = osb.tile([E + 1, LQ], F16, tag="osumB")
                    nc.scalar.copy(osumA[:], oaccA0[:])
                    nc.vector.tensor_copy(osumB[:], oaccB0[:])
                    q0sl = slice(q0 * LQ, (q0 + 1) * LQ)
                    nc.sync.dma_start(out=o[p0, 0][:, q0sl], in_=osumA[:])
                    nc.sync.dma_start(out=o[p0, 1][:, q0sl], in_=osumB[:])
                    del state[u0]
            if g < TOTAL:
                u, m = divmod(g, NBURST)
                p, q = divmod(u, NPASS)
                if q == 0 and m == 0:
                    tiles[p] = emit_pair_io(p)
                    if p >= 2:
                        del tiles[p - 2]
                qtd, kt, vp = tiles[p]
                if m == 0:
                    # two separate 1-bank tiles (NOT one [65,2,512]
                    # tile): oaccA accumulates the s 0:64 halves of
                    # every chunk, oaccB the s 64:128 halves.  Separate
                    # tiles let Tile prove the two drain copies read
                    # different PSUM banks, so the ScalarE copy and the
                    # VectorE copy run in parallel instead of being
                    # conservatively serialized.
                    oaccA = pacc.tile([E + 1, LQ], F32, tag="oaccA",
                                      name="oaccA")
                    oaccB = pacc.tile([E + 1, LQ], F32, tag="oaccB",
                                      name="oaccB")
                    state[u] = ((oaccA, oaccB), [None] * NCHUNK)
                _, ets = state[u]
                qsl = slice(q * LQ, (q + 1) * LQ)
                # QK: chunks 2m (rows 0-63) and 2m+1 (rows 64-127) run
                # concurrently, draining into the two banks of one
                # score tile.
                sc = pscore.tile([P, 2, LQ], F32, tag="score")
                for h in range(2):
                    lo, hi = (0, E) if h == 0 else (E, P)
                    nc.tensor.matmul(
                        sc[:, h, :], kt[lo:hi, m, :], qtd[lo:hi, qsl],
                        start=True, stop=True,
                        tile_position=(lo, 0),
                    )
                # exp of the whole burst tile in one FD=1024 op
                if g % 16 in DVE_BURSTS:
                    et = etp.tile([P, 2, LQ], I16, tag="eti")
                    nc.vector.tensor_scalar(
                        et[:], sc[:],
                        float(SA), float(SB),
                        mybir.AluOpType.mult, mybir.AluOpType.add,
                    )
                    et = et.bitcast(BF16)
                else:
                    et = etp.tile([P, 2, LQ], BF16, tag="etb")
                    nc.scalar.activation(et[:], sc[:], EXPF, scale=SCALE)
                ets[2 * m] = (et, 0)
                ets[2 * m + 1] = (et, 1)


_CACHE = {}


def _build():
    if "nc" in _CACHE:
        return _CACHE["nc"]
    nc = bacc.Bacc("TRN2", target_bir_lowering=False, debug=False,
                   num_devices=N_CORES)
    qt = nc.dram_tensor("qt", [PAIRS, E, L], BF16, kind="ExternalInput").ap()
    kt = nc.dram_tensor("kt", [PAIRS, 2, NBURST, E, P], BF16,
                        kind="ExternalInput").ap()
    v = nc.dram_tensor("v", [PAIRS, L, E + 1], BF16,
                       kind="ExternalInput").ap()
    qt0 = nc.dram_tensor("qt0", [P, L], BF16, kind="ExternalInput").ap()
    kt0 = nc.dram_tensor("kt0", [P, NBURST, P], BF16,
                         kind="ExternalInput").ap()
    v0 = nc.dram_tensor("v0", [P, NCHUNK, E + 1], BF16,
                        kind="ExternalInput").ap()
    o = nc.dram_tensor("o", [PAIRS, 2, E + 1, L], F16,
                       kind="ExternalOutput").ap()
    with tile.TileContext(nc) as tc:
        _attention(tc, o, qt, kt, v, qt0, kt0, v0)
    nc.compile()
    _CACHE["nc"] = nc
    return nc


def run(queries, keys, values, trace=False, **kw):
    """Run the SPMD kernel; returns (out_full, BassKernelResults)."""
    nc = _build()
    # [B, L, H, E] -> heads-major layouts the device DMAs straight in.
    qh = np.transpose(np.asarray(queries), (0, 2, 3, 1)).reshape(B * H, E, L)
    qh = np.ascontiguousarray(qh).astype(ml_dtypes.bfloat16)   # [64, E, L]
    kh = np.transpose(np.asarray(keys), (0, 2, 3, 1)).reshape(B * H, E, L)
    # [64, E, L] -> [64, 2, 8, E, 128]: half h gets s-chunks 2c+h
    kh = kh.reshape(B * H, E, NBURST, 2, P)
    kh = np.ascontiguousarray(np.transpose(kh, (0, 3, 2, 1, 4))).astype(
        ml_dtypes.bfloat16)
    vh = np.transpose(np.asarray(values), (0, 2, 1, 3)).reshape(B * H, L, E)
    vh1 = np.ones((B * H, L, E + 1), dtype=ml_dtypes.bfloat16)
    vh1[:, :, 0:E] = vh.astype(ml_dtypes.bfloat16)
    vh = vh1
    in_maps = []
    for c in range(N_CORES):
        q0 = np.concatenate([qh[c * PAIRS], qh[c * PAIRS]], axis=0)
        k0 = kh[c * PAIRS]                     # [2, 8, E, 128]
        k0 = np.ascontiguousarray(
            np.transpose(k0, (0, 2, 1, 3))).reshape(P, NBURST, P)
        w0 = vh[c * PAIRS].reshape(NCHUNK, P, E + 1)
        w0 = np.ascontiguousarray(np.transpose(w0, (1, 0, 2)))
        in_maps.append(
            {"qt": qh[c * PAIRS : (c + 1) * PAIRS],
             "kt": kh[c * PAIRS : (c + 1) * PAIRS],
             "v": vh[c * PAIRS : (c + 1) * PAIRS],
             "qt0": q0, "kt0": k0, "v0": w0})
    res = run_bass_kernel_spmd(nc, in_maps, list(range(N_CORES)),
                               trace=trace, **kw)
    # [64, 2, E+1, L] fp16: two partial accumulators per pair; rows 0..63
    # unnormalized O^T halves, row 64 the softmax-denominator halves.
    oh = np.concatenate([res.results[c]["o"] for c in range(N_CORES)], axis=0)
    oh = oh.astype(np.float32)
    osum = oh[:, 0] + oh[:, 1]                           # [64, 65, L]
    onorm = osum[:, 0:E, :] / osum[:, E : E + 1, :]      # softmax divide
    out = np.transpose(onorm.reshape(B, H, E, L), (0, 3, 1, 2))
    return np.ascontiguousarray(out), res


def kernel(queries, keys, values):
    out, _ = run(queries, keys, values)
    return out


# revision 62
# speedup vs baseline: 1.0084x; 1.0084x over previous
"""Full (non-causal) multi-head attention for Trainium2, 8-core SPMD.

Problem: B=4, L=2048, H=16, E=64 fp32.
  scores = einsum('blhe,bshe->bhls', Q, K) * 1/sqrt(E)
  attn   = softmax(scores, axis=-1)
  out    = einsum('bhls,bshd->blhd', attn, V)

Sharding: the 64 (b,h) pairs are split over 8 NeuronCores, 8 pairs per
core; attention is fully independent per (b,h), so no cross-core
communication.  The host hands each core Q^T/K^T/V already transposed /
cast to bf16, and takes back two unnormalized partial accumulators
O'_A/O'_B[65, l] per pair (each with a softmax-denominator ride-along
row), merging + dividing + final transpose on the host.

Per-core algorithm — one fully continuous global pipeline over all 32
(pair, l-pass) units (l in passes of 512, one PSUM bank of fp32):
  - Global burst g runs the QK+exp of unit g//8 (burst g%8 = s-chunks
    2m, 2m+1) and the AV matmuls of the unit DLAG=4 bursts back; the
    AV stream crosses pass and pair boundaries so the PE never drains.
  - Scores are computed transposed, S^T[s, l]: per burst one PSUM tile
    [128, 2, 512] (2 banks); the two QK matmuls (chunk 2m -> PE rows
    0-63, chunk 2m+1 -> rows 64-127, contraction E=64, tile_position
    row tiling) run concurrently and drain into the two banks.
  - exp() runs once per burst as a single FD=1024 instruction over the
    whole [128, 1024] score tile (amortizes the ~150-310-cycle fixed
    per-instruction engine overhead): ScalarE bursts use exact exp
    (activation, bf16 out), VectorE bursts a mean-centered Schraudolph
    bit-trick exp (i16 = round(score*A + B), bitcast bf16; sigma ~1.8%
    per element, zero-mean, common mode cancels in the softmax
    divide).  Bursts STRICTLY alternate engines (DVE odd, ACT even):
    both engines run at ~95% of the PE's 648ns/burst cadence, and any
    same-engine adjacency cascades into PE stalls through the 3-tile
    score ring.
  - AV is row-tiled like QK so every LDWEIGHTS in the kernel is a
    64-row load that background-loads behind the other row-half's
    streaming matmul (a full-array AV LDWEIGHTS cannot overlap
    row-tiled QK streams, which cost ~220ns per QK<->AV transition in
    the v1 layout): chunk a's AV is two concurrent matmuls,
    V'[s 0:64]^T E -> oacc[:,0,:] and V'[s 64:128]^T E -> oacc[:,1,:],
    accumulated over all 16 chunks.  V' carries a ones column so row
    64 of each accumulator half is that half's softmax-denominator
    partial; the host adds the halves.  Each pass's first two AV
    bursts are deferred two slots (burst 2 catches up) so the previous
    pass's drain copy clears the oacc WAR with ~2 bursts of slack.
  - PSUM budget (the binding constraint): 3 score tiles x 2 banks +
    oacc (2 banks) = 8 banks = all of PSUM.
  - Dummy warmup matmuls at kernel start keep the PE HAM activity
    window busy during the first DMA wait so real matmuls start at
    2.4 GHz instead of 1.2 GHz.
"""

import math

import numpy as np
import ml_dtypes
from contextlib import ExitStack

import concourse.bass as bass
import concourse.mybir as mybir
import concourse.tile as tile
from concourse import bacc
from concourse.bass_utils import run_bass_kernel_spmd

N_CORES = 8
B, L, H, E = 4, 2048, 16, 64
PAIRS = (B * H) // N_CORES    # 8 (b,h) pairs per core
P = 128                       # s-chunk size / partition count
NCHUNK = L // P               # 16 s-chunks
LQ = 512                      # l-quarter (one PSUM bank of fp32)
NPASS = L // LQ               # 4 passes over l per pair
NBURST = NCHUNK // 2          # 8 QK bursts (2 chunks each) per pass
SCALE = 1.0 / 8.0             # 1/sqrt(E)
DLAG = 4                      # AV stream trails QK stream by this many bursts
N_WARMUP = 48

F32 = mybir.dt.float32
F16 = mybir.dt.float16
BF16 = mybir.dt.bfloat16
I16 = mybir.dt.int16

# Schraudolph constants: i16 = round(score*SA + SB) bitcast bf16
# approximates exp(score/8) with zero-mean multiplicative error.
LOG2E = 1.4426950408889634
SA = 128.0 * LOG2E * SCALE
_fs = np.linspace(0.0, 1.0, 200001)[:-1]
_ECORR = float(np.mean((1.0 + _fs) * 2.0 ** (-_fs)))
SB = 127.0 * 128.0 - 128.0 * math.log2(_ECORR)

# Engine split, tuned so ScalarE and VectorE each stay under the PE's
# burst cadence: DVE handles global bursts with g % 16 in DVE_BURSTS
# (exp via Schraudolph; ~1221ns/burst vs ScalarE's ~1023), ScalarE the
# rest (exact exp).  18/14 per pair keeps both engines just under the
# PE's ~648ns/burst cadence including their half of the drain copies.
DVE_BURSTS = (1, 3, 5, 7, 9, 11, 13, 15)


def _attention(tc: tile.TileContext, o, qt, kt_d, v, qt0, kt0, v0):
    nc = tc.nc
    EXPF = mybir.ActivationFunctionType.Exp

    with ExitStack() as ctx:
        # PE warmup: dummy matmuls on a zero tile keep the HAM activity
        # window busy while the first pair's DMAs land.  wsb stays open for
        # the whole kernel: if it closed, the io pool would reuse its
        # released SBUF zone (stack allocator overlap-dep) and the input
        # DMAs would inherit a wait on all warmup matmuls.
        wsb = ctx.enter_context(tc.tile_pool(name="wsb", bufs=1))
        wz = wsb.tile([64, 64], BF16, tag="wz")
        nc.vector.memset(wz[:], 0.0)
        with tc.tile_pool(name="wps", bufs=1, space="PSUM") as wps:
            wp = wps.tile([64, 64], F32, tag="wp")
            for _ in range(N_WARMUP):
                nc.tensor.matmul(wp[:], wz[:], wz[:], start=True, stop=True)

        io = ctx.enter_context(tc.tile_pool(name="io", bufs=2))
        etp = ctx.enter_context(tc.tile_pool(name="etp", bufs=8))
        osb = ctx.enter_context(tc.tile_pool(name="osb", bufs=2))

        # PSUM: 3 score tiles x 2 banks + oaccA/oaccB x 1 bank = 8
        pscore = ctx.enter_context(tc.tile_pool(name="pscore", bufs=3, space="PSUM"))
        pacc = ctx.enter_context(tc.tile_pool(name="pacc", bufs=1, space="PSUM"))

        # ---- fully continuous global pipeline ----
        # One burst stream over all (pair, pass) units: global burst g
        # runs the QK+exp of unit g//8, burst g%8, and the AVs of the
        # unit DLAG bursts back.  The AV stream crosses pass and pair
        # boundaries, so the PE never drains at a boundary; the only
        # per-pass serialization is the oacc drain copy (split A/B
        # across both engines so the WAR clears in ~700ns).
        def emit_pair_io(p):
            # load Q^T (duplicated to both halves), K^T (paired), V.
            # The sync engine runs ahead of the PE, so these DMAs issue
            # as soon as the previous-previous pair's reads release the
            # io slot — mid-way through pair p-1.
            qtd = io.tile([P, L], BF16, tag="qtd")
            kt = io.tile([P, NBURST, P], BF16, tag="kt")
            vp = io.tile([P, NCHUNK, E + 1], BF16, tag="vp")
            # kt_d[p] is [2, 8, 64, 128]: half h holds chunks 2c+h.
            # Pair 0 loads from pre-staged contiguous copies (qt0/kt0/
            # v0) so the startup DMA is burst-friendly — safe only here
            # because no matmuls stream during it (contiguous DMA bursts
            # steal SBUF bandwidth from the PE); later pairs keep the
            # strided source patterns, which self-throttle.
            if p == 0:
                nc.sync.dma_start(out=qtd[:], in_=qt0)
                nc.sync.dma_start(out=kt[:], in_=kt0)
                nc.sync.dma_start(out=vp[:], in_=v0)
            else:
                nc.sync.dma_start(out=qtd[0:E, :], in_=qt[p])
                nc.sync.dma_start(out=qtd[E:P, :], in_=qt[p])
                nc.sync.dma_start(
                    out=kt[0:E, :, :],
                    in_=kt_d[p, 0].rearrange("c e l -> e c l"),
                )
                nc.sync.dma_start(
                    out=kt[E:P, :, :],
                    in_=kt_d[p, 1].rearrange("c e l -> e c l"),
                )
                # v[p] is [L, E+1], ones column pre-filled on the host (a
                # device memset would RMW-race the DMA in shared 4B words)
                nc.sync.dma_start(
                    out=vp[:], in_=v[p].rearrange("(c p) e -> p c e", p=P)
                )
            return qtd, kt, vp

        NUNIT = PAIRS * NPASS          # 32 (pair, pass) units
        TOTAL = NUNIT * NBURST         # 256 global bursts
        tiles = {}                     # pair -> (qtd, kt, vp)
        state = {}                     # unit -> (oacc, ets)
        for g in range(TOTAL + DLAG):
            # AV (lagged stream): chunk a = two concurrent row-tiled
            # matmuls accumulating into oacc[:,0,:] / [:,1,:].
            ga = g - DLAG
            if ga >= 0:
                u0, m0 = divmod(ga, NBURST)
                # defer each pass's first two AV bursts by two slots so
                # the previous pass's oacc drain copy gets ~2 bursts of
                # slack before the WAR write (AV a=0, start=True) lands;
                # burst 2 catches up with all three.  Total PE work per
                # pass is unchanged — only the order shifts.
                if m0 in (0, 1):
                    av_bursts = []
                elif m0 == 2:
                    av_bursts = [ga - 2, ga - 1, ga]
                else:
                    av_bursts = [ga]
            else:
                av_bursts = []
            for gaa in av_bursts:
                u0, m0 = divmod(gaa, NBURST)
                p0, q0 = divmod(u0, NPASS)
                (oaccA0, oaccB0), ets0 = state[u0]
                vp0 = tiles[p0][2]
                for a in (2 * m0, 2 * m0 + 1):
                    et, half = ets0[a]
                    nc.tensor.matmul(
                        oaccA0[:], vp0[0:E, a, :], et[0:E, half, :],
                        start=(a == 0), stop=(a == NCHUNK - 1),
                        tile_position=(0, 0),
                    )
                    nc.tensor.matmul(
                        oaccB0[:], vp0[E:P, a, :], et[E:P, half, :],
                        start=(a == 0), stop=(a == NCHUNK - 1),
                        tile_position=(E, 0),
                    )
                if m0 == NBURST - 1:
                    # drain the finished pass's accumulators: split A/B
                    # across both engines (separate tiles — one shared
                    # tile would add a false cross-engine WAW) so the
                    # WAR on oacc clears in one FD=512 copy (~700ns);
                    # fp16 halves the DMA bytes.  Host adds + divides.
                    osumA = osb.tile([E + 1, LQ], F16, tag="osumA")
                    osumB = osb.tile([E + 1, LQ], F16, tag="osumB")
                    nc.scalar.copy(osumA[:], oaccA0[:])
                    nc.vector.tensor_copy(osumB[:], oaccB0[:])
                    q0sl = slice(q0 * LQ, (q0 + 1) * LQ)
                    nc.sync.dma_start(out=o[p0, 0][:, q0sl], in_=osumA[:])
                    nc.sync.dma_start(out=o[p0, 1][:, q0sl], in_=osumB[:])
                    del state[u0]
            if g < TOTAL:
                u, m = divmod(g, NBURST)
                p, q = divmod(u, NPASS)
                if q == 0 and m == 0:
                    tiles[p] = emit_pair_io(p)
                    if p >= 2:
                        del tiles[p - 2]
                qtd, kt, vp = tiles[p]
                if m == 0:
                    # two separate 1-bank tiles (NOT one [65,2,512]
                    # tile): oaccA accumulates the s 0:64 halves of
                    # every chunk, oaccB the s 64:128 halves.  Separate
                    # tiles let Tile prove the two drain copies read
                    # different PSUM banks, so the ScalarE copy and the
                    # VectorE copy run in parallel instead of being
                    # conservatively serialized.
                    oaccA = pacc.tile([E + 1, LQ], F32, tag="oaccA",
                                      name="oaccA")
                    oaccB = pacc.tile([E + 1, LQ], F32, tag="oaccB",
                                      name="oaccB")
                    state[u] = ((oaccA, oaccB), [None] * NCHUNK)
                _, ets = state[u]
                qsl = slice(q * LQ, (q + 1) * LQ)
                # QK: chunks 2m (rows 0-63) and 2m+1 (rows 64-127) run
                # concurrently, draining into the two banks of one
                # score tile.
                sc = pscore.tile([P, 2, LQ], F32, tag="score")
                for h in range(2):
                    lo, hi = (0, E) if h == 0 else (E, P)
                    nc.tensor.matmul(
                        sc[:, h, :], kt[lo:hi, m, :], qtd[lo:hi, qsl],
                        start=True, stop=True,
                        tile_position=(lo, 0),
                    )
                # exp of the whole burst tile in one FD=1024 op
                if g % 16 in DVE_BURSTS:
                    et = etp.tile([P, 2, LQ], I16, tag="eti")
                    nc.vector.tensor_scalar(
                        et[:], sc[:],
                        float(SA), float(SB),
                        mybir.AluOpType.mult, mybir.AluOpType.add,
                    )
                    et = et.bitcast(BF16)
                else:
                    et = etp.tile([P, 2, LQ], BF16, tag="etb")
                    nc.scalar.activation(et[:], sc[:], EXPF, scale=SCALE)
                ets[2 * m] = (et, 0)
                ets[2 * m + 1] = (et, 1)


_CACHE = {}


def _build():
    if "nc" in _CACHE:
        return _CACHE["nc"]
    nc = bacc.Bacc("TRN2", target_bir_lowering=False, debug=False,
                   num_devices=N_CORES)
    qt = nc.dram_tensor("qt", [PAIRS, E, L], BF16, kind="ExternalInput").ap()
    kt = nc.dram_tensor("kt", [PAIRS, 2, NBURST, E, P], BF16,
                        kind="ExternalInput").ap()
    v = nc.dram_tensor("v", [PAIRS, L, E + 1], BF16,
                       kind="ExternalInput").ap()
    qt0 = nc.dram_tensor("qt0", [P, L], BF16, kind="ExternalInput").ap()
    kt0 = nc.dram_tensor("kt0", [P, NBURST, P], BF16,
                         kind="ExternalInput").ap()
    v0 = nc.dram_tensor("v0", [P, NCHUNK, E + 1], BF16,
                        kind="ExternalInput").ap()
    o = nc.dram_tensor("o", [PAIRS, 2, E + 1, L], F16,
                       kind="ExternalOutput").ap()
    with tile.TileContext(nc) as tc:
        _attention(tc, o, qt, kt, v, qt0, kt0, v0)
    nc.compile()
    _CACHE["nc"] = nc
    return nc


def run(queries, keys, values, trace=False, **kw):
    """Run the SPMD kernel; returns (out_full, BassKernelResults)."""
    nc = _build()
    # [B, L, H, E] -> heads-major layouts the device DMAs straight in.
    qh = np.transpose(np.asarray(queries), (0, 2, 3, 1)).reshape(B * H, E, L)
    qh = np.ascontiguousarray(qh).astype(ml_dtypes.bfloat16)   # [64, E, L]
    kh = np.transpose(np.asarray(keys), (0, 2, 3, 1)).reshape(B * H, E, L)
    # [64, E, L] -> [64, 2, 8, E, 128]: half h gets s-chunks 2c+h
    kh = kh.reshape(B * H, E, NBURST, 2, P)
    kh = np.ascontiguousarray(np.transpose(kh, (0, 3, 2, 1, 4))).astype(
        ml_dtypes.bfloat16)
    vh = np.transpose(np.asarray(values), (0, 2, 1, 3)).reshape(B * H, L, E)
    vh1 = np.ones((B * H, L, E + 1), dtype=ml_dtypes.bfloat16)
    vh1[:, :, 0:E] = vh.astype(ml_dtypes.bfloat16)
    vh = vh1
    in_maps = []
    for c in range(N_CORES):
        q0 = np.concatenate([qh[c * PAIRS], qh[c * PAIRS]], axis=0)
        k0 = kh[c * PAIRS]                     # [2, 8, E, 128]
        k0 = np.ascontiguousarray(
            np.transpose(k0, (0, 2, 1, 3))).reshape(P, NBURST, P)
        w0 = vh[c * PAIRS].reshape(NCHUNK, P, E + 1)
        w0 = np.ascontiguousarray(np.transpose(w0, (1, 0, 2)))
        in_maps.append(
            {"qt": qh[c * PAIRS : (c + 1) * PAIRS],
             "kt": kh[c * PAIRS : (c + 1) * PAIRS],
             "v": vh[c * PAIRS : (c + 1) * PAIRS],
             "qt0": q0, "kt0": k0, "v0": w0})
    res = run_bass_kernel_spmd(nc, in_maps, list(range(N_CORES)),
                               trace=trace, **kw)
    # [64, 2, E+1, L] fp16: two partial accumulators per pair; rows 0..63
    # unnormalized O^T halves, row 64 the softmax-denominator halves.
    oh = np.concatenate([res.results[c]["o"] for c in range(N_CORES)], axis=0)
    oh = oh.astype(np.float32)
    osum = oh[:, 0] + oh[:, 1]                           # [64, 65, L]
    onorm = osum[:, 0:E, :] / osum[:, E : E + 1, :]      # softmax divide
    out = np.transpose(onorm.reshape(B, H, E, L), (0, 3, 1, 2))
    return np.ascontiguousarray(out), res


def kernel(queries, keys, values):
    out, _ = run(queries, keys, values)
    return out


# revision 63
# speedup vs baseline: 1.0147x; 1.0062x over previous
"""Full (non-causal) multi-head attention for Trainium2, 8-core SPMD.

Problem: B=4, L=2048, H=16, E=64 fp32.
  scores = einsum('blhe,bshe->bhls', Q, K) * 1/sqrt(E)
  attn   = softmax(scores, axis=-1)
  out    = einsum('bhls,bshd->blhd', attn, V)

Sharding: the 64 (b,h) pairs are split over 8 NeuronCores, 8 pairs per
core; attention is fully independent per (b,h), so no cross-core
communication.  The host hands each core Q^T/K^T/V already transposed /
cast to bf16, and takes back two unnormalized partial accumulators
O'_A/O'_B[65, l] per pair (each with a softmax-denominator ride-along
row), merging + dividing + final transpose on the host.

Per-core algorithm — one fully continuous global pipeline over all 32
(pair, l-pass) units (l in passes of 512, one PSUM bank of fp32):
  - Global burst g runs the QK+exp of unit g//8 (burst g%8 = s-chunks
    2m, 2m+1) and the AV matmuls of the unit DLAG=4 bursts back; the
    AV stream crosses pass and pair boundaries so the PE never drains.
  - Scores are computed transposed, S^T[s, l]: per burst one PSUM tile
    [128, 2, 512] (2 banks); the two QK matmuls (chunk 2m -> PE rows
    0-63, chunk 2m+1 -> rows 64-127, contraction E=64, tile_position
    row tiling) run concurrently and drain into the two banks.
  - exp() runs once per burst as a single FD=1024 instruction over the
    whole [128, 1024] score tile (amortizes the ~150-310-cycle fixed
    per-instruction engine overhead): ScalarE bursts use exact exp
    (activation, bf16 out), VectorE bursts a mean-centered Schraudolph
    bit-trick exp (i16 = round(score*A + B), bitcast bf16; sigma ~1.8%
    per element, zero-mean, common mode cancels in the softmax
    divide).  Bursts STRICTLY alternate engines (DVE odd, ACT even):
    both engines run at ~95% of the PE's 648ns/burst cadence, and any
    same-engine adjacency cascades into PE stalls through the 3-tile
    score ring.
  - AV is row-tiled like QK so every LDWEIGHTS in the kernel is a
    64-row load that background-loads behind the other row-half's
    streaming matmul (a full-array AV LDWEIGHTS cannot overlap
    row-tiled QK streams, which cost ~220ns per QK<->AV transition in
    the v1 layout): chunk a's AV is two concurrent matmuls,
    V'[s 0:64]^T E -> oacc[:,0,:] and V'[s 64:128]^T E -> oacc[:,1,:],
    accumulated over all 16 chunks.  V' carries a ones column so row
    64 of each accumulator half is that half's softmax-denominator
    partial; the host adds the halves.  Each pass's first two AV
    bursts are deferred two slots (burst 2 catches up) so the previous
    pass's drain copy clears the oacc WAR with ~2 bursts of slack.
  - PSUM budget (the binding constraint): 3 score tiles x 2 banks +
    oacc (2 banks) = 8 banks = all of PSUM.
  - Dummy warmup matmuls at kernel start keep the PE HAM activity
    window busy during the first DMA wait so real matmuls start at
    2.4 GHz instead of 1.2 GHz.
"""

import math

import numpy as np
import ml_dtypes
from contextlib import ExitStack

import concourse.bass as bass
import concourse.mybir as mybir
import concourse.tile as tile
from concourse import bacc
from concourse.bass_utils import run_bass_kernel_spmd

N_CORES = 8
B, L, H, E = 4, 2048, 16, 64
PAIRS = (B * H) // N_CORES    # 8 (b,h) pairs per core
P = 128                       # s-chunk size / partition count
NCHUNK = L // P               # 16 s-chunks
LQ = 512                      # l-quarter (one PSUM bank of fp32)
NPASS = L // LQ               # 4 passes over l per pair
NBURST = NCHUNK // 2          # 8 QK bursts (2 chunks each) per pass
SCALE = 1.0 / 8.0             # 1/sqrt(E)
DLAG = 4                      # AV stream trails QK stream by this many bursts
N_WARMUP = 48

F32 = mybir.dt.float32
F16 = mybir.dt.float16
BF16 = mybir.dt.bfloat16
I16 = mybir.dt.int16

# Schraudolph constants: i16 = round(score*SA + SB) bitcast bf16
# approximates exp(score/8) with zero-mean multiplicative error.
LOG2E = 1.4426950408889634
SA = 128.0 * LOG2E * SCALE
_fs = np.linspace(0.0, 1.0, 200001)[:-1]
_ECORR = float(np.mean((1.0 + _fs) * 2.0 ** (-_fs)))
SB = 127.0 * 128.0 - 128.0 * math.log2(_ECORR)

# Engine split, tuned so ScalarE and VectorE each stay under the PE's
# burst cadence: DVE handles global bursts with g % 16 in DVE_BURSTS
# (exp via Schraudolph; ~1221ns/burst vs ScalarE's ~1023), ScalarE the
# rest (exact exp).  18/14 per pair keeps both engines just under the
# PE's ~648ns/burst cadence including their half of the drain copies.
DVE_BURSTS = (1, 3, 5, 7, 9, 11, 13, 15)


def _attention(tc: tile.TileContext, o, qt, kt_d, v, qt0, kt0, v0):
    nc = tc.nc
    EXPF = mybir.ActivationFunctionType.Exp

    with ExitStack() as ctx:
        # PE warmup: dummy matmuls on a zero tile keep the HAM activity
        # window busy while the first pair's DMAs land.  wsb stays open for
        # the whole kernel: if it closed, the io pool would reuse its
        # released SBUF zone (stack allocator overlap-dep) and the input
        # DMAs would inherit a wait on all warmup matmuls.
        wsb = ctx.enter_context(tc.tile_pool(name="wsb", bufs=1))
        wz = wsb.tile([64, 64], BF16, tag="wz")
        nc.vector.memset(wz[:], 0.0)
        with tc.tile_pool(name="wps", bufs=1, space="PSUM") as wps:
            wp = wps.tile([64, 64], F32, tag="wp")
            for _ in range(N_WARMUP):
                nc.tensor.matmul(wp[:], wz[:], wz[:], start=True, stop=True)

        io = ctx.enter_context(tc.tile_pool(name="io", bufs=2))
        etp = ctx.enter_context(tc.tile_pool(name="etp", bufs=8))
        osb = ctx.enter_context(tc.tile_pool(name="osb", bufs=1))

        # PSUM: 3 score tiles x 2 banks + oaccA/oaccB x 1 bank = 8
        pscore = ctx.enter_context(tc.tile_pool(name="pscore", bufs=3, space="PSUM"))
        pacc = ctx.enter_context(tc.tile_pool(name="pacc", bufs=1, space="PSUM"))

        # ---- fully continuous global pipeline ----
        # One burst stream over all (pair, pass) units: global burst g
        # runs the QK+exp of unit g//8, burst g%8, and the AVs of the
        # unit DLAG bursts back.  The AV stream crosses pass and pair
        # boundaries, so the PE never drains at a boundary; the only
        # per-pass serialization is the oacc drain copy (split A/B
        # across both engines so the WAR clears in ~700ns).
        def emit_pair_io(p):
            # load Q^T (duplicated to both halves), K^T (paired), V.
            # The sync engine runs ahead of the PE, so these DMAs issue
            # as soon as the previous-previous pair's reads release the
            # io slot — mid-way through pair p-1.
            qtd = io.tile([P, L], BF16, tag="qtd")
            kt = io.tile([P, NBURST, P], BF16, tag="kt")
            vp = io.tile([P, NCHUNK, E + 1], BF16, tag="vp")
            # kt_d[p] is [2, 8, 64, 128]: half h holds chunks 2c+h.
            # Pair 0 loads from pre-staged contiguous copies (qt0/kt0/
            # v0) so the startup DMA is burst-friendly — safe only here
            # because no matmuls stream during it (contiguous DMA bursts
            # steal SBUF bandwidth from the PE); later pairs keep the
            # strided source patterns, which self-throttle.
            if p == 0:
                nc.sync.dma_start(out=qtd[:], in_=qt0)
                nc.sync.dma_start(out=kt[:], in_=kt0)
                nc.sync.dma_start(out=vp[:], in_=v0)
            else:
                nc.sync.dma_start(out=qtd[0:E, :], in_=qt[p])
                nc.sync.dma_start(out=qtd[E:P, :], in_=qt[p])
                nc.sync.dma_start(
                    out=kt[0:E, :, :],
                    in_=kt_d[p, 0].rearrange("c e l -> e c l"),
                )
                nc.sync.dma_start(
                    out=kt[E:P, :, :],
                    in_=kt_d[p, 1].rearrange("c e l -> e c l"),
                )
                # v[p] is [L, E+1], ones column pre-filled on the host (a
                # device memset would RMW-race the DMA in shared 4B words)
                nc.sync.dma_start(
                    out=vp[:], in_=v[p].rearrange("(c p) e -> p c e", p=P)
                )
            return qtd, kt, vp

        NUNIT = PAIRS * NPASS          # 32 (pair, pass) units
        TOTAL = NUNIT * NBURST         # 256 global bursts
        tiles = {}                     # pair -> (qtd, kt, vp)
        state = {}                     # unit -> (oacc, ets)
        for g in range(TOTAL + DLAG):
            # AV (lagged stream): chunk a = two concurrent row-tiled
            # matmuls accumulating into oacc[:,0,:] / [:,1,:].
            ga = g - DLAG
            if ga >= 0:
                u0, m0 = divmod(ga, NBURST)
                # defer each pass's first two AV bursts by two slots so
                # the previous pass's oacc drain copy gets ~2 bursts of
                # slack before the WAR write (AV a=0, start=True) lands;
                # burst 2 catches up with all three.  Total PE work per
                # pass is unchanged — only the order shifts.
                if m0 in (0, 1):
                    av_bursts = []
                elif m0 == 2:
                    av_bursts = [ga - 2, ga - 1, ga]
                else:
                    av_bursts = [ga]
            else:
                av_bursts = []
            for gaa in av_bursts:
                u0, m0 = divmod(gaa, NBURST)
                p0, q0 = divmod(u0, NPASS)
                (oaccA0, oaccB0), ets0 = state[u0]
                vp0 = tiles[p0][2]
                for a in (2 * m0, 2 * m0 + 1):
                    et, half = ets0[a]
                    nc.tensor.matmul(
                        oaccA0[:], vp0[0:E, a, :], et[0:E, half, :],
                        start=(a == 0), stop=(a == NCHUNK - 1),
                        tile_position=(0, 0),
                    )
                    nc.tensor.matmul(
                        oaccB0[:], vp0[E:P, a, :], et[E:P, half, :],
                        start=(a == 0), stop=(a == NCHUNK - 1),
                        tile_position=(E, 0),
                    )
                if m0 == NBURST - 1:
                    # drain the finished pass's accumulators: split A/B
                    # across both engines (separate tiles — one shared
                    # tile would add a false cross-engine WAW) so the
                    # WAR on oacc clears in one FD=512 copy (~700ns);
                    # fp16 halves the DMA bytes.  Host adds + divides.
                    osumA = osb.tile([E + 1, LQ], F16, tag="osumA")
                    osumB = osb.tile([E + 1, LQ], F16, tag="osumB")
                    nc.scalar.copy(osumA[:], oaccA0[:])
                    nc.vector.tensor_copy(osumB[:], oaccB0[:])
                    q0sl = slice(q0 * LQ, (q0 + 1) * LQ)
                    nc.sync.dma_start(out=o[p0, 0][:, q0sl], in_=osumA[:])
                    nc.sync.dma_start(out=o[p0, 1][:, q0sl], in_=osumB[:])
                    del state[u0]
            if g < TOTAL:
                u, m = divmod(g, NBURST)
                p, q = divmod(u, NPASS)
                if q == 0 and m == 0:
                    tiles[p] = emit_pair_io(p)
                    if p >= 2:
                        del tiles[p - 2]
                qtd, kt, vp = tiles[p]
                if m == 0:
                    # two separate 1-bank tiles (NOT one [65,2,512]
                    # tile): oaccA accumulates the s 0:64 halves of
                    # every chunk, oaccB the s 64:128 halves.  Separate
                    # tiles let Tile prove the two drain copies read
                    # different PSUM banks, so the ScalarE copy and the
                    # VectorE copy run in parallel instead of being
                    # conservatively serialized.
                    oaccA = pacc.tile([E + 1, LQ], F32, tag="oaccA",
                                      name="oaccA")
                    oaccB = pacc.tile([E + 1, LQ], F32, tag="oaccB",
                                      name="oaccB")
                    state[u] = ((oaccA, oaccB), [None] * NCHUNK)
                _, ets = state[u]
                qsl = slice(q * LQ, (q + 1) * LQ)
                # QK: chunks 2m (rows 0-63) and 2m+1 (rows 64-127) run
                # concurrently, draining into the two banks of one
                # score tile.
                sc = pscore.tile([P, 2, LQ], F32, tag="score")
                for h in range(2):
                    lo, hi = (0, E) if h == 0 else (E, P)
                    nc.tensor.matmul(
                        sc[:, h, :], kt[lo:hi, m, :], qtd[lo:hi, qsl],
                        start=True, stop=True,
                        tile_position=(lo, 0),
                    )
                # exp of the whole burst tile in one FD=1024 op
                if g % 16 in DVE_BURSTS:
                    et = etp.tile([P, 2, LQ], I16, tag="eti")
                    nc.vector.tensor_scalar(
                        et[:], sc[:],
                        float(SA), float(SB),
                        mybir.AluOpType.mult, mybir.AluOpType.add,
                    )
                    et = et.bitcast(BF16)
                else:
                    et = etp.tile([P, 2, LQ], BF16, tag="etb")
                    nc.scalar.activation(et[:], sc[:], EXPF, scale=SCALE)
                ets[2 * m] = (et, 0)
                ets[2 * m + 1] = (et, 1)


_CACHE = {}


def _build():
    if "nc" in _CACHE:
        return _CACHE["nc"]
    nc = bacc.Bacc("TRN2", target_bir_lowering=False, debug=False,
                   num_devices=N_CORES)
    qt = nc.dram_tensor("qt", [PAIRS, E, L], BF16, kind="ExternalInput").ap()
    kt = nc.dram_tensor("kt", [PAIRS, 2, NBURST, E, P], BF16,
                        kind="ExternalInput").ap()
    v = nc.dram_tensor("v", [PAIRS, L, E + 1], BF16,
                       kind="ExternalInput").ap()
    qt0 = nc.dram_tensor("qt0", [P, L], BF16, kind="ExternalInput").ap()
    kt0 = nc.dram_tensor("kt0", [P, NBURST, P], BF16,
                         kind="ExternalInput").ap()
    v0 = nc.dram_tensor("v0", [P, NCHUNK, E + 1], BF16,
                        kind="ExternalInput").ap()
    o = nc.dram_tensor("o", [PAIRS, 2, E + 1, L], F16,
                       kind="ExternalOutput").ap()
    with tile.TileContext(nc) as tc:
        _attention(tc, o, qt, kt, v, qt0, kt0, v0)
    nc.compile()
    _CACHE["nc"] = nc
    return nc


def run(queries, keys, values, trace=False, **kw):
    """Run the SPMD kernel; returns (out_full, BassKernelResults)."""
    nc = _build()
    # [B, L, H, E] -> heads-major layouts the device DMAs straight in.
    qh = np.transpose(np.asarray(queries), (0, 2, 3, 1)).reshape(B * H, E, L)
    qh = np.ascontiguousarray(qh).astype(ml_dtypes.bfloat16)   # [64, E, L]
    kh = np.transpose(np.asarray(keys), (0, 2, 3, 1)).reshape(B * H, E, L)
    # [64, E, L] -> [64, 2, 8, E, 128]: half h gets s-chunks 2c+h
    kh = kh.reshape(B * H, E, NBURST, 2, P)
    kh = np.ascontiguousarray(np.transpose(kh, (0, 3, 2, 1, 4))).astype(
        ml_dtypes.bfloat16)
    vh = np.transpose(np.asarray(values), (0, 2, 1, 3)).reshape(B * H, L, E)
    vh1 = np.ones((B * H, L, E + 1), dtype=ml_dtypes.bfloat16)
    vh1[:, :, 0:E] = vh.astype(ml_dtypes.bfloat16)
    vh = vh1
    in_maps = []
    for c in range(N_CORES):
        q0 = np.concatenate([qh[c * PAIRS], qh[c * PAIRS]], axis=0)
        k0 = kh[c * PAIRS]                     # [2, 8, E, 128]
        k0 = np.ascontiguousarray(
            np.transpose(k0, (0, 2, 1, 3))).reshape(P, NBURST, P)
        w0 = vh[c * PAIRS].reshape(NCHUNK, P, E + 1)
        w0 = np.ascontiguousarray(np.transpose(w0, (1, 0, 2)))
        in_maps.append(
            {"qt": qh[c * PAIRS : (c + 1) * PAIRS],
             "kt": kh[c * PAIRS : (c + 1) * PAIRS],
             "v": vh[c * PAIRS : (c + 1) * PAIRS],
             "qt0": q0, "kt0": k0, "v0": w0})
    res = run_bass_kernel_spmd(nc, in_maps, list(range(N_CORES)),
                               trace=trace, **kw)
    # [64, 2, E+1, L] fp16: two partial accumulators per pair; rows 0..63
    # unnormalized O^T halves, row 64 the softmax-denominator halves.
    oh = np.concatenate([res.results[c]["o"] for c in range(N_CORES)], axis=0)
    oh = oh.astype(np.float32)
    osum = oh[:, 0] + oh[:, 1]                           # [64, 65, L]
    onorm = osum[:, 0:E, :] / osum[:, E : E + 1, :]      # softmax divide
    out = np.transpose(onorm.reshape(B, H, E, L), (0, 3, 1, 2))
    return np.ascontiguousarray(out), res


def kernel(queries, keys, values):
    out, _ = run(queries, keys, values)
    return out
